# revision 3
# baseline (speedup 1.0000x reference)
"""Trainium2 Bass kernel for a dual-stream cross-attention block.

Reference computation (per batch element b, all fp32 in the oracle):
  Q_l = dwconv3(x_l @ lp1_w1^T + lp1_b1, lp1_w2) + lp1_b2   (and likewise
  Q_r/rp1, V_l/lp2, V_r/rp2)
  attn = Q_l @ Q_r^T * C^-0.5                               (T x T)
  F_r2l = softmax(attn, -1) @ V_r ;  F_l2r = softmax(attn, 1)^T... (bsc)
  out_l = x_l + F_r2l @ lp3_w^T + lp3_b
  out_r = x_r + F_l2r @ rp3_w^T + rp3_b

Sharding: data-parallel over B across the 8 cores (one batch element per
core), params replicated.  Inside a core everything is blocked for the
128x128 PE array; matmul inputs are bf16 (fp32 PSUM accumulation), the
residual/epilogue path stays fp32.
"""

import sys

for _p in ("/opt/trn_rl_repo",):
    if _p not in sys.path:
        sys.path.insert(0, _p)

from contextlib import ExitStack

import numpy as np

import concourse.bacc as bacc
import concourse.tile as tile
from concourse import mybir
from concourse.bass_utils import run_bass_kernel_spmd
from concourse.masks import make_identity

B, T, C = 8, 2048, 512
P = 128
NCORES = 8
CCH = C // P      # 4 feature chunks of 128
TCH = T // P      # 16 sequence chunks of 128
NT = 512          # moving-operand tile (free dim)
TT = T // NT      # 4 sequence tiles of 512
SCALE = float(C) ** -0.5

F32 = mybir.dt.float32
BF16 = mybir.dt.bfloat16
AX = mybir.AxisListType.X
MULT = mybir.AluOpType.mult
ADD = mybir.AluOpType.add
EXP = mybir.ActivationFunctionType.Exp

WNAMES = [
    "lp1_w1", "lp1_b1", "lp1_w2", "lp1_b2",
    "rp1_w1", "rp1_b1", "rp1_w2", "rp1_b2",
    "lp2_w1", "lp2_b1", "lp2_w2", "lp2_b2",
    "rp2_w1", "rp2_b1", "rp2_w2", "rp2_b2",
    "lp3_w", "lp3_b", "rp3_w", "rp3_b",
]


def _build_body(nc, tc, io, ctx):
    """Emit the per-core program.  io maps dram tensor name -> AP."""
    x_l, x_r = io["x_l"], io["x_r"]
    out_l, out_r = io["out_l"], io["out_r"]

    # ---------------- pools (persistent across the kernel) ----------------
    consts = ctx.enter_context(tc.tile_pool(name="consts", bufs=1))
    wp = ctx.enter_context(tc.tile_pool(name="wp", bufs=1))
    qv = ctx.enter_context(tc.tile_pool(name="qv", bufs=1))
    zp = ctx.enter_context(tc.tile_pool(name="zp", bufs=1))
    zstp = ctx.enter_context(tc.tile_pool(name="zstp", bufs=4))
    xload = ctx.enter_context(tc.tile_pool(name="xload", bufs=4))
    ps_mm = ctx.enter_context(tc.tile_pool(name="ps_mm", bufs=4, space="PSUM"))

    ident = consts.tile([P, P], F32)
    make_identity(nc, ident)
    ident_bf = consts.tile([P, P], BF16)
    make_identity(nc, ident_bf)
    ones_row = consts.tile([1, P], F32)
    nc.vector.memset(ones_row, 1.0)

    # broadcast final biases to all partitions: bc[p, d] = b[d]
    b3bc = {}
    for nm in ("lp3_b", "rp3_b"):
        b3row = consts.tile([1, C], F32, name=f"{nm}_row")
        nc.sync.dma_start(b3row, io[nm].rearrange("(a b) -> a b", a=1))
        pb = ps_mm.tile([P, C], F32, tag="mm", name=f"{nm}_ps")
        nc.tensor.matmul(pb, ones_row, b3row, start=True, stop=True)
        bc = consts.tile([P, C], F32, name=f"{nm}_bc")
        nc.vector.tensor_copy(bc, pb)
        b3bc[nm] = bc

    # per-projection small params: b1, b2 as [P, CCH]; w2 taps as [P, CCH, 3]
    small = {}
    for pj in ("lp1", "rp1", "lp2", "rp2"):
        b1t = consts.tile([P, CCH], F32, name=f"{pj}_b1t")
        b2t = consts.tile([P, CCH], F32, name=f"{pj}_b2t")
        w2t = consts.tile([P, CCH, 3], F32, name=f"{pj}_w2t")
        b1ap = io[f"{pj}_b1"].rearrange("(a b) -> a b", b=1)
        b2ap = io[f"{pj}_b2"].rearrange("(a b) -> a b", b=1)
        for ch in range(CCH):
            sl = slice(ch * P, (ch + 1) * P)
            nc.sync.dma_start(b1t[:, ch : ch + 1], b1ap[sl, :])
            nc.sync.dma_start(b2t[:, ch : ch + 1], b2ap[sl, :])
            nc.sync.dma_start(w2t[:, ch, :], io[f"{pj}_w2"][sl, :])
        small[pj] = (b1t, b2t, w2t)

    # persistent big tensors
    w3lT = wp.tile([P, CCH, C], BF16)   # lp3_w^T  [c, d]
    w3rT = wp.tile([P, CCH, C], BF16)
    QlT = qv.tile([P, CCH, T], BF16)    # Q^T feature-major [c, t]
    QrT = qv.tile([P, CCH, T], BF16)
    Vl = qv.tile([P, TCH, C], BF16)     # V natural [t, c]
    Vr = qv.tile([P, TCH, C], BF16)
    Z1 = zp.tile([P, TCH], F32)
    Z2 = zp.tile([P, TCH], F32)
    rZ1 = zp.tile([P, TCH], F32)
    rZ2 = zp.tile([P, TCH], F32)

    # ---------------- phase 0/1: weights, transposes, projections ----------
    with ExitStack() as p1:
        wstage = p1.enter_context(tc.tile_pool(name="wstage", bufs=2))
        w1p = p1.enter_context(tc.tile_pool(name="w1p", bufs=1))
        xtp = p1.enter_context(tc.tile_pool(name="xtp", bufs=1))
        hp = p1.enter_context(tc.tile_pool(name="hp", bufs=2))
        vfmp = p1.enter_context(tc.tile_pool(name="vfmp", bufs=1))
        ps_tr = p1.enter_context(tc.tile_pool(name="ps_tr", bufs=3, space="PSUM"))

        def load_wT(dst, w_ap):
            # dst[p, ci, dj*P + j] = w[dj*P + j, ci*P + p]
            for dj in range(CCH):
                wn = wstage.tile([P, C], F32, tag="wstage", name="wn")
                nc.sync.dma_start(wn, w_ap[dj * P : (dj + 1) * P, :])
                for ci in range(CCH):
                    pt = ps_tr.tile([P, P], F32, tag="ptr", name="ptw")
                    nc.tensor.transpose(pt, wn[:, ci * P : (ci + 1) * P], ident)
                    nc.vector.tensor_copy(dst[:, ci, dj * P : (dj + 1) * P], pt)

        w1T = {}
        for pj in ("lp1", "rp1", "lp2", "rp2"):
            w1T[pj] = w1p.tile([P, CCH, C], BF16, name=f"{pj}_w1T")
            load_wT(w1T[pj], io[f"{pj}_w1"])
        load_wT(w3lT, io["lp3_w"])
        load_wT(w3rT, io["rp3_w"])

        def load_xT(dst, x_ap):
            # dst[p, ci, tc*P + j] = x[tc*P + j, ci*P + p]
            for tcn in range(TCH):
                xn = xload.tile([P, C], F32, tag="xl", name="xn")
                nc.sync.dma_start(xn, x_ap[tcn * P : (tcn + 1) * P, :])
                for ci in range(CCH):
                    pt = ps_tr.tile([P, P], F32, tag="ptr", name="ptx")
                    nc.tensor.transpose(pt, xn[:, ci * P : (ci + 1) * P], ident)
                    nc.vector.tensor_copy(dst[:, ci, tcn * P : (tcn + 1) * P], pt)

        def project(dst, xT, pj):
            """dst[:, dc, t] = depthwise-conv3(x @ w1^T + b1)^T in [d, t]."""
            b1t, b2t, w2t = small[pj]
            H = hp.tile([P, CCH, T], BF16, tag="H", name=f"H_{pj}")
            for tt in range(TT):
                tsl = slice(tt * NT, (tt + 1) * NT)
                for dc in range(CCH):
                    ph = ps_mm.tile([P, NT], F32, tag="mm", name="ph")
                    for cc in range(CCH):
                        nc.tensor.matmul(
                            ph,
                            w1T[pj][:, cc, dc * P : (dc + 1) * P],
                            xT[:, cc, tsl],
                            start=(cc == 0),
                            stop=(cc == CCH - 1),
                        )
                    nc.scalar.add(H[:, dc, tsl], ph, b1t[:, dc : dc + 1])
            for dc in range(CCH):
                h = H[:, dc, :]
                q = dst[:, dc, :]
                # center tap + bias, then the two shifted taps accumulated
                nc.vector.tensor_scalar(
                    q, h, w2t[:, dc, 1:2], b2t[:, dc : dc + 1], op0=MULT, op1=ADD
                )
                nc.vector.scalar_tensor_tensor(
                    q[:, 1:T], h[:, 0 : T - 1], w2t[:, dc, 0:1], q[:, 1:T],
                    op0=MULT, op1=ADD,
                )
                nc.vector.scalar_tensor_tensor(
                    q[:, 0 : T - 1], h[:, 1:T], w2t[:, dc, 2:3], q[:, 0 : T - 1],
                    op0=MULT, op1=ADD,
                )

        def v_to_natural(vnat, vfm):
            # vnat[p, sc, ci*P + j] = vfm[j, ci, sc*P + p]
            for sc in range(TCH):
                for ci in range(CCH):
                    pt = ps_tr.tile([P, P], BF16, tag="ptr", name="ptv")
                    nc.tensor.transpose(pt, vfm[:, ci, sc * P : (sc + 1) * P], ident_bf)
                    nc.scalar.copy(vnat[:, sc, ci * P : (ci + 1) * P], pt)

        xT = xtp.tile([P, CCH, T], BF16, tag="xT", name="xlT")
        load_xT(xT, x_l)
        project(QlT, xT, "lp1")
        VlT = vfmp.tile([P, CCH, T], BF16, tag="vfm", name="VlT")
        project(VlT, xT, "lp2")
        v_to_natural(Vl, VlT)

        xT = xtp.tile([P, CCH, T], BF16, tag="xT", name="xrT")
        load_xT(xT, x_r)
        project(QrT, xT, "rp1")
        VrT = vfmp.tile([P, CCH, T], BF16, tag="vfm", name="VrT")
        project(VrT, xT, "rp2")
        v_to_natural(Vr, VrT)

    # ---------------- phases 2/3: attention ----------------
    ep = ctx.enter_context(tc.tile_pool(name="ep", bufs=1))
    u2p = ctx.enter_context(tc.tile_pool(name="u2p", bufs=1))
    gp = ctx.enter_context(tc.tile_pool(name="gp", bufs=2))
    tmpp = ctx.enter_context(tc.tile_pool(name="tmpp", bufs=3))
    outp = ctx.enter_context(tc.tile_pool(name="outp", bufs=3))
    ps_u = ctx.enter_context(tc.tile_pool(name="ps_u", bufs=2, space="PSUM"))

    def s_pass(E, Z, qrow, qcol):
        """E[:, rc, s] = exp(scale * qrow^T qcol), Z[:, rc] = row sums."""
        for rc in range(TCH):
            zst = zstp.tile([P, TT], F32, tag="zst", name="zst")
            for st in range(TT):
                ssl = slice(st * NT, (st + 1) * NT)
                ps = ps_mm.tile([P, NT], F32, tag="mm", name="ps_s")
                for cc in range(CCH):
                    nc.tensor.matmul(
                        ps,
                        qrow[:, cc, rc * P : (rc + 1) * P],
                        qcol[:, cc, ssl],
                        start=(cc == 0),
                        stop=(cc == CCH - 1),
                    )
                nc.scalar.activation(
                    E[:, rc, ssl], ps, EXP, scale=SCALE,
                    accum_out=zst[:, st : st + 1],
                )
            nc.vector.reduce_sum(Z[:, rc : rc + 1], zst, axis=AX)

    def pv_u(E, V, w3T, sink):
        """G = V^T E (contracting the chunk axis), U = G^T w3T, sink(idx, U)."""
        for st in range(TT):
            tsl = slice(st * NT, (st + 1) * NT)
            G = gp.tile([P, CCH, NT], BF16, tag="G", name="G")
            for cc in range(CCH):
                pg = ps_mm.tile([P, NT], F32, tag="mm", name="pg")
                for kc in range(TCH):
                    nc.tensor.matmul(
                        pg,
                        V[:, kc, cc * P : (cc + 1) * P],
                        E[:, kc, tsl],
                        start=(kc == 0),
                        stop=(kc == TCH - 1),
                    )
                nc.scalar.copy(G[:, cc, :], pg)
            for sb in range(TT):
                pu = ps_u.tile([P, NT], F32, tag="pu", name="pu")
                for cc in range(CCH):
                    nc.tensor.matmul(
                        pu,
                        G[:, cc, sb * P : (sb + 1) * P],
                        w3T[:, cc, :],
                        start=(cc == 0),
                        stop=(cc == CCH - 1),
                    )
                sink(st * TT + sb, pu)

    # E1 in [t, s] layout (+ Z1), consumed by the l->r direction
    E = ep.tile([P, TCH, T], BF16, tag="E", name="E1")
    s_pass(E, Z1, QlT, QrT)
    nc.vector.reciprocal(rZ1, Z1)

    # l->r direction: G2 = V_l^T E1, U2 = G2^T rp3_w^T -> stash (Z2 not yet known)
    U2st = u2p.tile([P, TCH, C], BF16)

    def sink_stash(idx, pu):
        nc.scalar.copy(U2st[:, idx, :], pu)

    pv_u(E, Vl, w3rT, sink_stash)

    # E2 in [s, t] layout (+ Z2); reuses E1's slot (same tag, bufs=1)
    E = ep.tile([P, TCH, T], BF16, tag="E", name="E2")
    s_pass(E, Z2, QrT, QlT)
    nc.vector.reciprocal(rZ2, Z2)

    # out_r epilogue from the stash
    for sc in range(TCH):
        rsl = slice(sc * P, (sc + 1) * P)
        xr = xload.tile([P, C], F32, tag="xl", name="xr_ep")
        nc.sync.dma_start(xr, x_r[rsl, :])
        tmp = tmpp.tile([P, C], F32, tag="tmp", name="tmp_r")
        nc.gpsimd.tensor_add(tmp, xr, b3bc["rp3_b"])
        o = outp.tile([P, C], F32, tag="o", name="o_r")
        nc.vector.scalar_tensor_tensor(
            o, U2st[:, sc, :], rZ2[:, sc : sc + 1], tmp, op0=MULT, op1=ADD
        )
        nc.sync.dma_start(out_r[rsl, :], o)

    # r->l direction: G1 = V_r^T E2, U1 = G1^T lp3_w^T -> direct epilogue
    def sink_l(idx, pu):
        rsl = slice(idx * P, (idx + 1) * P)
        xl = xload.tile([P, C], F32, tag="xl", name="xl_ep")
        nc.sync.dma_start(xl, x_l[rsl, :])
        tmp = tmpp.tile([P, C], F32, tag="tmp", name="tmp_l")
        nc.gpsimd.tensor_add(tmp, xl, b3bc["lp3_b"])
        o = outp.tile([P, C], F32, tag="o", name="o_l")
        nc.vector.scalar_tensor_tensor(
            o, pu, rZ1[:, idx : idx + 1], tmp, op0=MULT, op1=ADD
        )
        nc.sync.dma_start(out_l[rsl, :], o)

    pv_u(E, Vr, w3lT, sink_l)


def build_nc():
    nc = bacc.Bacc(
        "TRN2",
        target_bir_lowering=False,
        debug=False,
        enable_asserts=False,
        num_devices=NCORES,
    )
    io = {}
    io["x_l"] = nc.dram_tensor("x_l", [T, C], F32, kind="ExternalInput").ap()
    io["x_r"] = nc.dram_tensor("x_r", [T, C], F32, kind="ExternalInput").ap()
    for nm in WNAMES:
        if nm.endswith("_w1") or nm in ("lp3_w", "rp3_w"):
            shape = [C, C]
        elif nm.endswith("_w2"):
            shape = [C, 3]
        else:
            shape = [C]
        io[nm] = nc.dram_tensor(nm, shape, F32, kind="ExternalInput").ap()
    io["out_l"] = nc.dram_tensor("out_l", [T, C], F32, kind="ExternalOutput").ap()
    io["out_r"] = nc.dram_tensor("out_r", [T, C], F32, kind="ExternalOutput").ap()

    with tile.TileContext(nc) as tc:
        with ExitStack() as ctx:
            _build_body(nc, tc, io, ctx)
    nc.compile()
    return nc


_NC_CACHE = None


def _get_nc():
    global _NC_CACHE
    if _NC_CACHE is None:
        _NC_CACHE = build_nc()
    return _NC_CACHE


def make_in_maps(inputs):
    ins = {k: np.ascontiguousarray(np.asarray(v, dtype=np.float32)) for k, v in inputs.items()}
    in_maps = []
    for c in range(NCORES):
        m = {"x_l": ins["x_l"][c], "x_r": ins["x_r"][c]}
        for nm in WNAMES:
            m[nm] = ins[nm]
        in_maps.append(m)
    return in_maps


def run(inputs, **kw):
    nc = _get_nc()
    res = run_bass_kernel_spmd(nc, make_in_maps(inputs), list(range(NCORES)), **kw)
    out_l = np.stack([res.results[c]["out_l"] for c in range(NCORES)])
    out_r = np.stack([res.results[c]["out_r"] for c in range(NCORES)])
    return (out_l, out_r), res


def kernel(**inputs):
    outs, _ = run(inputs)
    return outs


# revision 9
# speedup vs baseline: 1.0247x; 1.0247x over previous
"""Trainium2 Bass kernel for a dual-stream cross-attention block.

Reference computation (per batch element b, all fp32 in the oracle):
  Q_l = dwconv3(x_l @ lp1_w1^T + lp1_b1, lp1_w2) + lp1_b2   (and likewise
  Q_r/rp1, V_l/lp2, V_r/rp2)
  attn = Q_l @ Q_r^T * C^-0.5                               (T x T)
  F_r2l = softmax(attn, -1) @ V_r ;  F_l2r = softmax(attn, 1)^T... (bsc)
  out_l = x_l + F_r2l @ lp3_w^T + lp3_b
  out_r = x_r + F_l2r @ rp3_w^T + rp3_b

Sharding: data-parallel over B across the 8 cores (one batch element per
core), params replicated.  Inside a core everything is blocked for the
128x128 PE array; matmul inputs are bf16 (fp32 PSUM accumulation), the
residual/epilogue path stays fp32.
"""

import sys

for _p in ("/opt/trn_rl_repo",):
    if _p not in sys.path:
        sys.path.insert(0, _p)

from contextlib import ExitStack

import numpy as np

import concourse.bacc as bacc
import concourse.tile as tile
from concourse import mybir
from concourse.bass_utils import run_bass_kernel_spmd
from concourse.masks import make_identity

B, T, C = 8, 2048, 512
P = 128
NCORES = 8
CCH = C // P      # 4 feature chunks of 128
TCH = T // P      # 16 sequence chunks of 128
NT = 512          # moving-operand tile (free dim)
TT = T // NT      # 4 sequence tiles of 512
SCALE = float(C) ** -0.5

F32 = mybir.dt.float32
BF16 = mybir.dt.bfloat16
FP8 = mybir.dt.float8e4
AX = mybir.AxisListType.X
MULT = mybir.AluOpType.mult
ADD = mybir.AluOpType.add
EXP = mybir.ActivationFunctionType.Exp

WNAMES = [
    "lp1_w1", "lp1_b1", "lp1_w2", "lp1_b2",
    "rp1_w1", "rp1_b1", "rp1_w2", "rp1_b2",
    "lp2_w1", "lp2_b1", "lp2_w2", "lp2_b2",
    "rp2_w1", "rp2_b1", "rp2_w2", "rp2_b2",
    "lp3_w", "lp3_b", "rp3_w", "rp3_b",
]


def _build_body(nc, tc, io, ctx):
    """Emit the per-core program.  io maps dram tensor name -> AP."""
    x_l, x_r = io["x_l"], io["x_r"]
    out_l, out_r = io["out_l"], io["out_r"]

    # ---------------- pools (persistent across the kernel) ----------------
    consts = ctx.enter_context(tc.tile_pool(name="consts", bufs=1))
    wp = ctx.enter_context(tc.tile_pool(name="wp", bufs=1))
    qv = ctx.enter_context(tc.tile_pool(name="qv", bufs=1))
    zp = ctx.enter_context(tc.tile_pool(name="zp", bufs=1))
    zstp = ctx.enter_context(tc.tile_pool(name="zstp", bufs=2))
    xload = ctx.enter_context(tc.tile_pool(name="xload", bufs=4))
    ps_mm = ctx.enter_context(tc.tile_pool(name="ps_mm", bufs=4, space="PSUM"))

    ident = consts.tile([P, P], F32)
    make_identity(nc, ident)
    ident_bf = consts.tile([P, P], BF16)
    make_identity(nc, ident_bf)
    ones_row = consts.tile([1, P], F32)
    nc.vector.memset(ones_row, 1.0)

    def load_small_params():
        # one strided DMA per tensor: [c] -> [p, chunk], [c, 3] -> [p, chunk, 3]
        small = {}
        for pj in ("lp1", "rp1", "lp2", "rp2"):
            b1t = consts.tile([P, CCH], F32, name=f"{pj}_b1t")
            b2t = consts.tile([P, CCH], F32, name=f"{pj}_b2t")
            w2t = consts.tile([P, CCH, 3], F32, name=f"{pj}_w2t")
            nc.sync.dma_start(b1t, io[f"{pj}_b1"].rearrange("(a b) -> b a", a=CCH))
            nc.sync.dma_start(b2t, io[f"{pj}_b2"].rearrange("(a b) -> b a", a=CCH))
            nc.sync.dma_start(w2t, io[f"{pj}_w2"].rearrange("(a b) c -> b a c", a=CCH))
            small[pj] = (b1t, b2t, w2t)
        return small

    def load_b3bc():
        # broadcast final biases to all partitions: bc[p, d] = b[d]
        b3bc = {}
        for nm in ("lp3_b", "rp3_b"):
            b3row = consts.tile([1, C], F32, name=f"{nm}_row")
            nc.sync.dma_start(b3row, io[nm].rearrange("(a b) -> a b", a=1))
            pb = ps_mm.tile([P, C], F32, tag="mm", name=f"{nm}_ps")
            nc.tensor.matmul(pb, ones_row, b3row, start=True, stop=True)
            bc = consts.tile([P, C], F32, name=f"{nm}_bc")
            nc.vector.tensor_copy(bc, pb)
            b3bc[nm] = bc
        return b3bc

    # persistent big tensors
    w3lT = wp.tile([P, CCH, C], BF16)   # lp3_w^T  [c, d]
    w3rT = wp.tile([P, CCH, C], BF16)
    QlT = qv.tile([P, CCH, T], BF16)    # Q^T feature-major [c, t]
    QrT = qv.tile([P, CCH, T], BF16)
    Vl = qv.tile([P, TCH, C], FP8)      # V natural [t, c]
    Vr = qv.tile([P, TCH, C], FP8)
    Z1 = zp.tile([P, TCH], F32)
    Z2 = zp.tile([P, TCH], F32)
    rZ1 = zp.tile([P, TCH], F32)
    rZ2 = zp.tile([P, TCH], F32)

    # ---------------- phase 0/1: weights, transposes, projections ----------
    with ExitStack() as p1:
        wstage = p1.enter_context(tc.tile_pool(name="wstage", bufs=2))
        w1p = p1.enter_context(tc.tile_pool(name="w1p", bufs=1))
        xtp = p1.enter_context(tc.tile_pool(name="xtp", bufs=2))
        hp = p1.enter_context(tc.tile_pool(name="hp", bufs=2))
        vfmp = p1.enter_context(tc.tile_pool(name="vfmp", bufs=1))
        ps_tr = p1.enter_context(tc.tile_pool(name="ps_tr", bufs=3, space="PSUM"))

        def load_wT(dst, w_ap):
            # dst[p, ci, dj*P + j] = w[dj*P + j, ci*P + p]
            for dj in range(CCH):
                wn = wstage.tile([P, C], F32, tag="wstage", name="wn")
                nc.sync.dma_start(wn, w_ap[dj * P : (dj + 1) * P, :])
                for ci in range(CCH):
                    pt = ps_tr.tile([P, P], F32, tag="ptr", name="ptw")
                    nc.tensor.transpose(pt, wn[:, ci * P : (ci + 1) * P], ident)
                    nc.vector.tensor_copy(dst[:, ci, dj * P : (dj + 1) * P], pt)

        w1T = {}
        for pj in ("lp1", "rp1", "lp2", "rp2"):
            w1T[pj] = w1p.tile([P, CCH, C], BF16, name=f"{pj}_w1T")
            load_wT(w1T[pj], io[f"{pj}_w1"])

        def load_xT(dst, x_ap):
            # dst[p, ci, tc*P + j] = x[tc*P + j, ci*P + p]
            for tcn in range(TCH):
                xn = xload.tile([P, C], F32, tag="xl", name="xn")
                nc.sync.dma_start(xn, x_ap[tcn * P : (tcn + 1) * P, :])
                for ci in range(CCH):
                    pt = ps_tr.tile([P, P], F32, tag="ptr", name="ptx")
                    nc.tensor.transpose(pt, xn[:, ci * P : (ci + 1) * P], ident)
                    nc.vector.tensor_copy(dst[:, ci, tcn * P : (tcn + 1) * P], pt)

        def project(dst, xT, pj):
            """dst[:, dc, t] = depthwise-conv3(x @ w1^T + b1)^T in [d, t]."""
            b1t, b2t, w2t = small[pj]
            H = hp.tile([P, CCH, T], BF16, tag="H", name=f"H_{pj}")
            for tt in range(TT):
                tsl = slice(tt * NT, (tt + 1) * NT)
                for dc in range(CCH):
                    ph = ps_mm.tile([P, NT], F32, tag="mm", name="ph")
                    for cc in range(CCH):
                        nc.tensor.matmul(
                            ph,
                            w1T[pj][:, cc, dc * P : (dc + 1) * P],
                            xT[:, cc, tsl],
                            start=(cc == 0),
                            stop=(cc == CCH - 1),
                        )
                    nc.scalar.add(H[:, dc, tsl], ph, b1t[:, dc : dc + 1])
            for dc in range(CCH):
                h = H[:, dc, :]
                q = dst[:, dc, :]
                # center tap + bias, then the two shifted taps accumulated
                nc.vector.tensor_scalar(
                    q, h, w2t[:, dc, 1:2], b2t[:, dc : dc + 1], op0=MULT, op1=ADD
                )
                nc.vector.scalar_tensor_tensor(
                    q[:, 1:T], h[:, 0 : T - 1], w2t[:, dc, 0:1], q[:, 1:T],
                    op0=MULT, op1=ADD,
                )
                nc.vector.scalar_tensor_tensor(
                    q[:, 0 : T - 1], h[:, 1:T], w2t[:, dc, 2:3], q[:, 0 : T - 1],
                    op0=MULT, op1=ADD,
                )

        def v_to_natural(vnat, vfm):
            # vnat[p, sc, ci*P + j] = vfm[j, ci, sc*P + p]
            for sc in range(TCH):
                for ci in range(CCH):
                    pt = ps_tr.tile([P, P], BF16, tag="ptr", name="ptv")
                    nc.tensor.transpose(pt, vfm[:, ci, sc * P : (sc + 1) * P], ident_bf)
                    nc.scalar.copy(vnat[:, sc, ci * P : (ci + 1) * P], pt)

        # Q projections first so the attention score pass unblocks as early
        # as possible; V projections + their transposes fill in behind it.
        xlT = xtp.tile([P, CCH, T], BF16, tag="xT", name="xlT")
        load_xT(xlT, x_l)
        xrT = xtp.tile([P, CCH, T], BF16, tag="xT", name="xrT")
        load_xT(xrT, x_r)
        small = load_small_params()
        b3bc = load_b3bc()
        load_wT(w3lT, io["lp3_w"])
        load_wT(w3rT, io["rp3_w"])
        project(QlT, xlT, "lp1")
        project(QrT, xrT, "rp1")
        VlT = vfmp.tile([P, CCH, T], BF16, tag="vfm", name="VlT")
        project(VlT, xlT, "lp2")
        v_to_natural(Vl, VlT)
        VrT = vfmp.tile([P, CCH, T], BF16, tag="vfm", name="VrT")
        project(VrT, xrT, "rp2")
        v_to_natural(Vr, VrT)

    # ---------------- phases 2/3: attention ----------------
    ep1 = ctx.enter_context(tc.tile_pool(name="ep1", bufs=1))
    ep2 = ctx.enter_context(tc.tile_pool(name="ep2", bufs=1))
    u2p = ctx.enter_context(tc.tile_pool(name="u2p", bufs=1))
    gp = ctx.enter_context(tc.tile_pool(name="gp", bufs=2))
    tmpp = ctx.enter_context(tc.tile_pool(name="tmpp", bufs=3))
    outp = ctx.enter_context(tc.tile_pool(name="outp", bufs=3))
    ps_u = ctx.enter_context(tc.tile_pool(name="ps_u", bufs=2, space="PSUM"))

    def s_pass(E, Z, qrow, qcol):
        """E[:, rc, s] = exp(scale * qrow^T qcol), Z[:, rc] = row sums.

        Emitted column-major (st outer) so each score column is complete
        early and the downstream PV accumulation can start behind it."""
        zst = zstp.tile([P, TCH, TT], F32, tag="zst", name="zst")
        for st in range(TT):
            ssl = slice(st * NT, (st + 1) * NT)
            for rc in range(TCH):
                ps = ps_mm.tile([P, NT], F32, tag="mm", name="ps_s")
                for cc in range(CCH):
                    nc.tensor.matmul(
                        ps,
                        qrow[:, cc, rc * P : (rc + 1) * P],
                        qcol[:, cc, ssl],
                        start=(cc == 0),
                        stop=(cc == CCH - 1),
                    )
                nc.scalar.activation(
                    E[:, rc, ssl], ps, EXP, scale=SCALE,
                    accum_out=zst[:, rc, st : st + 1],
                )
        nc.vector.reduce_sum(Z, zst, axis=AX)

    def pv_u(E, V, w3T, sink):
        """G = V^T E (contracting the chunk axis), U = G^T w3T, sink(idx, U)."""
        for st in range(TT):
            tsl = slice(st * NT, (st + 1) * NT)
            G = gp.tile([P, CCH, NT], BF16, tag="G", name="G")
            for cc in range(CCH):
                pg = ps_mm.tile([P, NT], F32, tag="mm", name="pg")
                for kc2 in range(TCH // 2):
                    nc.tensor.matmul(
                        pg,
                        V[:, 2 * kc2 : 2 * kc2 + 2, cc * P : (cc + 1) * P],
                        E[:, 2 * kc2 : 2 * kc2 + 2, tsl],
                        start=(kc2 == 0),
                        stop=(kc2 == TCH // 2 - 1),
                        perf_mode=mybir.MatmulPerfMode.DoubleRow,
                    )
                nc.scalar.copy(G[:, cc, :], pg)
            for sb in range(TT):
                pu = ps_u.tile([P, NT], F32, tag="pu", name="pu")
                for cc in range(CCH):
                    nc.tensor.matmul(
                        pu,
                        G[:, cc, sb * P : (sb + 1) * P],
                        w3T[:, cc, :],
                        start=(cc == 0),
                        stop=(cc == CCH - 1),
                    )
                sink(st * TT + sb, pu)

    # E1 in [t, s] layout (+ Z1), consumed by the l->r direction
    E1 = ep1.tile([P, TCH, T], FP8, name="E1")
    s_pass(E1, Z1, QlT, QrT)
    nc.vector.reciprocal(rZ1, Z1)

    # l->r direction: G2 = V_l^T E1, U2 = G2^T rp3_w^T -> stash (Z2 not yet known)
    U2st = u2p.tile([P, TCH, C], BF16)

    def sink_stash(idx, pu):
        nc.scalar.copy(U2st[:, idx, :], pu)

    pv_u(E1, Vl, w3rT, sink_stash)

    # E2 in [s, t] layout (+ Z2), independent slot so phases overlap freely
    E2 = ep2.tile([P, TCH, T], FP8, name="E2")
    s_pass(E2, Z2, QrT, QlT)
    nc.vector.reciprocal(rZ2, Z2)

    # out_r epilogue from the stash
    for sc in range(TCH):
        rsl = slice(sc * P, (sc + 1) * P)
        xr = xload.tile([P, C], F32, tag="xl", name="xr_ep")
        nc.sync.dma_start(xr, x_r[rsl, :])
        tmp = tmpp.tile([P, C], F32, tag="tmp", name="tmp_r")
        nc.gpsimd.tensor_add(tmp, xr, b3bc["rp3_b"])
        o = outp.tile([P, C], F32, tag="o", name="o_r")
        nc.vector.scalar_tensor_tensor(
            o, U2st[:, sc, :], rZ2[:, sc : sc + 1], tmp, op0=MULT, op1=ADD
        )
        nc.sync.dma_start(out_r[rsl, :], o)

    # r->l direction: G1 = V_r^T E2, U1 = G1^T lp3_w^T -> direct epilogue
    def sink_l(idx, pu):
        rsl = slice(idx * P, (idx + 1) * P)
        xl = xload.tile([P, C], F32, tag="xl", name="xl_ep")
        nc.sync.dma_start(xl, x_l[rsl, :])
        tmp = tmpp.tile([P, C], F32, tag="tmp", name="tmp_l")
        nc.gpsimd.tensor_add(tmp, xl, b3bc["lp3_b"])
        o = outp.tile([P, C], F32, tag="o", name="o_l")
        nc.vector.scalar_tensor_tensor(
            o, pu, rZ1[:, idx : idx + 1], tmp, op0=MULT, op1=ADD
        )
        nc.sync.dma_start(out_l[rsl, :], o)

    pv_u(E2, Vr, w3lT, sink_l)


def build_nc():
    nc = bacc.Bacc(
        "TRN2",
        target_bir_lowering=False,
        debug=False,
        enable_asserts=False,
        num_devices=NCORES,
    )
    io = {}
    io["x_l"] = nc.dram_tensor("x_l", [T, C], F32, kind="ExternalInput").ap()
    io["x_r"] = nc.dram_tensor("x_r", [T, C], F32, kind="ExternalInput").ap()
    for nm in WNAMES:
        if nm.endswith("_w1") or nm in ("lp3_w", "rp3_w"):
            shape = [C, C]
        elif nm.endswith("_w2"):
            shape = [C, 3]
        else:
            shape = [C]
        io[nm] = nc.dram_tensor(nm, shape, F32, kind="ExternalInput").ap()
    io["out_l"] = nc.dram_tensor("out_l", [T, C], F32, kind="ExternalOutput").ap()
    io["out_r"] = nc.dram_tensor("out_r", [T, C], F32, kind="ExternalOutput").ap()

    with tile.TileContext(nc) as tc:
        with ExitStack() as ctx:
            _build_body(nc, tc, io, ctx)
    nc.compile()
    return nc


_NC_CACHE = None


def _get_nc():
    global _NC_CACHE
    if _NC_CACHE is None:
        _NC_CACHE = build_nc()
    return _NC_CACHE


def make_in_maps(inputs):
    ins = {k: np.ascontiguousarray(np.asarray(v, dtype=np.float32)) for k, v in inputs.items()}
    in_maps = []
    for c in range(NCORES):
        m = {"x_l": ins["x_l"][c], "x_r": ins["x_r"][c]}
        for nm in WNAMES:
            m[nm] = ins[nm]
        in_maps.append(m)
    return in_maps


def run(inputs, **kw):
    nc = _get_nc()
    res = run_bass_kernel_spmd(nc, make_in_maps(inputs), list(range(NCORES)), **kw)
    out_l = np.stack([res.results[c]["out_l"] for c in range(NCORES)])
    out_r = np.stack([res.results[c]["out_r"] for c in range(NCORES)])
    return (out_l, out_r), res


def kernel(**inputs):
    outs, _ = run(inputs)
    return outs
